# revision 48
# baseline (speedup 1.0000x reference)
"""Trainium2 Bass kernel for nn_Extract_HyperSpherePrototypes.

Computation (see reference):
  1. L2-normalize each pixel's feature vector over the channel dim F=256.
  2. Segment-sum normalized features by label into [C+1=20, F] prototypes.
  3. Drop void class, transpose to [F, 19], L2-normalize each column.

Sharding: data-parallel over batch (16 items / 8 cores = 2 per core).
Each core computes a local [20, 256] partial, AllReduce(sum) across the
8 cores, then every core normalizes columns and writes the full output.

Schedule notes (cost-model driven):
  - Per-queue DMA transfers serialize and block that engine's instruction
    stream, but different queues (SP / ACT / Pool-SWDGE) overlap fully.
    Compute engines carry DMAs only in windows they would idle anyway:
    b0 rides SP (fc0 + fc3-low half), ACT (fc1) and Pool (fc3-high half,
    fc2); b1fc0 preloads into the spare 5th buffer, and b1's remaining
    tiles reuse b0's buffers as the wq3 matmuls retire them.
  - Per (fc, wq=32-col chunk): squares into fp16 (split across ACT, Pool
    and DVE by tile-landing time), DVE pairwise-add tree (fp16
    tensor_tensor runs at 2x; some level-1 adds offloaded to Pool) into a
    [H, NFC, 4, W] arena, one strided-XY tensor_reduce per wq, ACT sqrt
    (fp16, table preloaded via a dummy op), DVE reciprocal.
  - DVE builds onehot(label) in fp16 early (labels only; Pool lacks
    is_equal); Pool multiplies it by inv into an f32r mask (the PE
    rejects mixed 32/16-bit matmul inputs and its f32r input must be
    written as f32r for rounding). The tensor engine contracts h with
    QW=4 w-columns packed per matmul (moving operand N=256 keeps f32r at
    1 cycle/row); cross (wl != wl') PSUM blocks are dropped by the
    diagonal-block combine, which reads one PSUM operand per op (engine
    constraint) and overlaps the last matmul groups.
"""

import numpy as np

import concourse.bass as bass
import concourse.bacc as bacc
from concourse import mybir
from concourse.bass_utils import run_bass_kernel_spmd
from concourse.tile import TileContext

F32 = mybir.dt.float32
F32R = mybir.dt.float32r
FP16 = mybir.dt.float16
AX = mybir.AxisListType
OP = mybir.AluOpType
ACT_FN = mybir.ActivationFunctionType

NCORES = 8
B_TOT = 16
BPC = B_TOT // NCORES  # batches per core
F = 256
H = 128
W = 128
C = 20  # 19 known + void
FC = 64  # f-chunk per tile
NFC = F // FC
WQ = 32  # w-chunk for compute pipeline
NWQ = W // WQ
QW = 4  # w-columns packed per matmul (lhsT = [h, QW*CP])
CP = 32  # class block padded to PSUM partition alignment

EPS2 = 1e-24  # matches max(norm, 1e-12) in the reference

_NO_CC = False

# square-chunk engine per (fc, wq): "A" = ACT, "P" = Pool, "D" = DVE.
# b0: fc2 lands last (Pool 2nd transfer) -> fc2 chunks on Pool (free
# right then); fc0/fc1 wq3 balance Pool late.
SQ_ENG_B0 = {
    (0, 0): "A", (1, 0): "A", (2, 0): "P", (3, 0): "A",
    (0, 1): "A", (1, 1): "A", (2, 1): "P", (3, 1): "A",
    (0, 2): "A", (1, 2): "A", (2, 2): "P", (3, 2): "A",
    (0, 3): "P", (1, 3): "P", (2, 3): "P", (3, 3): "A",
}
# b1: fc0 lands early; fc2/fc1/fc3 land ~55-58 -> spread their wq0
# chunks across DVE/ACT/Pool so the last M-chain compresses.
SQ_ENG_B1 = {
    (0, 0): "A", (1, 0): "A", (2, 0): "D", (3, 0): "P",
    (0, 1): "A", (1, 1): "A", (2, 1): "P", (3, 1): "A",
    (0, 2): "A", (1, 2): "A", (2, 2): "P", (3, 2): "A",
    (0, 3): "P", (1, 3): "P", (2, 3): "A", (3, 3): "A",
}
# tree level-1 adds offloaded to Pool for late chunks (DVE decongestion)
POOL_L1_B0 = {(0, 2), (1, 2), (0, 3), (1, 3)}
POOL_L1_B1 = {(0, 2), (1, 2), (0, 3), (1, 3)}
# emission order of square chunks per batch (wq-major):
SQ_ORDER = [
    (0, 0), (1, 0), (2, 0), (3, 0),
    (0, 1), (1, 1), (2, 1), (3, 1),
    (0, 2), (1, 2), (2, 2), (3, 2),
    (0, 3), (1, 3), (2, 3), (3, 3),
]


def build_nc():
    # masks must be f32r: the PE rejects mixed 32/16-bit matmul inputs
    m_dt = F32R
    nc = bacc.Bacc("TRN2", target_bir_lowering=False)

    feats = nc.declare_dram_parameter("feats", [BPC, F, H, W], F32R, isOutput=False)
    labs = nc.declare_dram_parameter("labs", [BPC, H, W], FP16, isOutput=False)
    out_d = nc.declare_dram_parameter("out", [C - 1, F], F32, isOutput=True)

    cc_in = nc.dram_tensor("cc_in", [C, F], F32)
    cc_out = nc.dram_tensor("cc_out", [C, F], F32, addr_space="Shared")

    with TileContext(nc) as tc:
        with (
            tc.tile_pool(name="consts", bufs=1) as consts,
            tc.tile_pool(name="xp", bufs=5) as xp,
            tc.tile_pool(name="sqp", bufs=3) as sqp,
            tc.tile_pool(name="t1p", bufs=2) as t1p,
            tc.tile_pool(name="t2p", bufs=2) as t2p,
            tc.tile_pool(name="t3p", bufs=2) as t3p,
            tc.tile_pool(name="arp", bufs=2) as arp,
            tc.tile_pool(name="ohp", bufs=4) as ohp,
            tc.tile_pool(name="mp", bufs=2) as mp,
            tc.tile_pool(name="normp", bufs=2) as normp,
            tc.tile_pool(name="finp", bufs=1) as finp,
            tc.tile_pool(name="psum", bufs=1, space="PSUM") as psum,
        ):
            iota_i = consts.tile([H, CP], mybir.dt.int32)
            nc.gpsimd.iota(iota_i, pattern=[[1, CP]], base=0, channel_multiplier=0)
            iota_sb = consts.tile([H, CP], FP16)
            nc.vector.tensor_copy(iota_sb, iota_i)
            eps_sb = consts.tile([H, 1], F32)
            nc.vector.memset(eps_sb, EPS2)
            eps16 = consts.tile([H, 1], FP16)
            nc.vector.memset(eps16, 6e-8)
            # dummy sqrt: force the ACT function-table load off the
            # critical path (the set covers Square too)
            warm = consts.tile([H, 1], F32)
            nc.scalar.activation(out=warm, in_=eps_sb, func=ACT_FN.Sqrt,
                                 bias=eps_sb[:])

            feats_ap = feats.ap()
            labs_ap = labs.ap()

            psq = []
            for fc in range(NFC):
                psq_t = psum.tile([QW * CP, FC * QW], F32, name=f"psq{fc}")
                psq.append(psq_t)

            lab_sbs = []
            for b in range(BPC):
                lab_sb = normp.tile([H, W], FP16, name="lab_sb")
                nc.sync.dma_start(out=lab_sb, in_=labs_ap[b])
                lab_sbs.append(lab_sb)

            xts = {}

            def emit_dma(b, fc, eng=None, half=None):
                xt = xts[(b, fc)]
                hfw = feats_ap[b].rearrange("f h w -> h f w")
                if half is None:
                    eng.dma_start(out=xt, in_=hfw[:, fc * FC : (fc + 1) * FC, :])
                else:
                    lo, hi = (0, FC // 2) if half == 0 else (FC // 2, FC)
                    eng.dma_start(
                        out=xt[:, lo:hi, :],
                        in_=hfw[:, fc * FC + lo : fc * FC + hi, :],
                    )

            def emit_oh(b):
                lab_sb = lab_sbs[b]
                ohs = []
                for wq in range(NWQ):
                    ws = slice(wq * WQ, (wq + 1) * WQ)
                    oh = ohp.tile([H, WQ, CP], FP16, name="oh")
                    nc.vector.tensor_tensor(
                        out=oh,
                        in0=bass.AP(
                            tensor=iota_sb[:].tensor,
                            offset=iota_sb[:].offset,
                            ap=[iota_sb[:].ap[0], [0, WQ], [1, CP]],
                        ),
                        in1=lab_sb[:, ws].to_broadcast([H, WQ, CP]),
                        op=OP.is_equal,
                    )
                    ohs.append(oh)
                return ohs

            def emit_batch(b, ohs, sq_eng):
                arena = arp.tile([H, NFC, 4, W], FP16, name="arena")
                ssq = normp.tile([H, W], FP16, name="ssq")
                inv = normp.tile([H, W], FP16, name="inv")
                done_tree = set()

                def sq_tree(fc, wq):
                    ws = slice(wq * WQ, (wq + 1) * WQ)
                    sq = sqp.tile([H, FC, WQ], FP16, name="sq")
                    src = xts[(b, fc)][:, :, ws].bitcast(F32)
                    eng = sq_eng[(fc, wq)]
                    if eng == "A":
                        nc.scalar.activation(out=sq, in_=src, func=ACT_FN.Square)
                    elif eng == "P":
                        nc.gpsimd.tensor_tensor(out=sq, in0=src, in1=src, op=OP.mult)
                    else:
                        nc.vector.tensor_tensor(out=sq, in0=src, in1=src, op=OP.mult)
                    t1 = t1p.tile([H, FC // 2, WQ], FP16, name="t1")
                    l1_eng = nc.gpsimd if (fc, wq) in (POOL_L1_B0 if b == 0 else POOL_L1_B1) else nc.vector
                    l1_eng.tensor_tensor(
                        out=t1, in0=sq[:, 0::2, :], in1=sq[:, 1::2, :], op=OP.add
                    )
                    t2 = t2p.tile([H, FC // 4, WQ], FP16, name="t2")
                    nc.vector.tensor_tensor(
                        out=t2, in0=t1[:, 0::2, :], in1=t1[:, 1::2, :], op=OP.add
                    )
                    t3 = t3p.tile([H, FC // 8, WQ], FP16, name="t3")
                    nc.vector.tensor_tensor(
                        out=t3, in0=t2[:, 0::2, :], in1=t2[:, 1::2, :], op=OP.add
                    )
                    nc.vector.tensor_tensor(
                        out=arena[:, fc, :, ws],
                        in0=t3[:, 0::2, :],
                        in1=t3[:, 1::2, :],
                        op=OP.add,
                    )
                    done_tree.add((fc, wq))

                def norm_mask_mm(wq):
                    ws = slice(wq * WQ, (wq + 1) * WQ)
                    # sum the 16 per-fc partials in one strided XY reduce
                    nc.vector.tensor_reduce(
                        out=ssq[:, ws],
                        in_=arena[:, :, :, ws].rearrange("h n e w -> h w n e"),
                        axis=AX.XY,
                        op=OP.add,
                    )
                    nc.scalar.activation(
                        out=ssq[:, ws], in_=ssq[:, ws], func=ACT_FN.Sqrt,
                        bias=eps16[:],
                    )
                    nc.vector.reciprocal(out=inv[:, ws], in_=ssq[:, ws])
                    m_sb = mp.tile([H, WQ, CP], m_dt, name="m_sb")
                    nc.gpsimd.tensor_tensor(
                        out=m_sb[:],
                        in0=ohs[wq][:],
                        in1=inv[:, ws].to_broadcast([H, WQ, CP]),
                        op=OP.mult,
                    )
                    for fc in range(NFC):
                        for ql in range(WQ // QW):
                            nc.tensor.matmul(
                                out=psq[fc],
                                lhsT=m_sb[:, ql * QW : (ql + 1) * QW, :].rearrange(
                                    "h w c -> h (w c)"
                                ),
                                rhs=xts[(b, fc)][
                                    :, :, wq * WQ + ql * QW : wq * WQ + (ql + 1) * QW
                                ],
                                start=(b == 0 and wq == 0 and ql == 0),
                                stop=(
                                    b == BPC - 1
                                    and wq == NWQ - 1
                                    and ql == WQ // QW - 1
                                ),
                            )

                emitted_norm = set()
                for fc, wq in SQ_ORDER:
                    sq_tree(fc, wq)
                    for w_ready in range(NWQ):
                        if w_ready in emitted_norm:
                            continue
                        if all((f, w_ready) in done_tree for f in range(NFC)):
                            norm_mask_mm(w_ready)
                            emitted_norm.add(w_ready)

            with nc.allow_low_precision(reason="fp16 sumsq tree and masks"):
                # canonical slot rotation: b0fc0..3 + b1fc0 take the 5
                # buffers; b1fc3/fc2/fc1 (created in that order) reuse
                # b0fc0/fc1/fc2's buffers, which free in that order when
                # b0's wq3 matmuls retire fc-major.
                for fc in range(NFC):
                    xts[(0, fc)] = xp.tile([H, FC, W], F32R, name="xt")
                xts[(1, 0)] = xp.tile([H, FC, W], F32R, name="xt")
                # b0 queues: SP fc0 + fc3lo, ACT fc1, Pool fc3hi + fc2
                emit_dma(0, 0, nc.sync)
                emit_dma(0, 1, nc.scalar)
                emit_dma(0, 3, nc.gpsimd, half=1)
                emit_dma(0, 3, nc.sync, half=0)
                emit_dma(0, 2, nc.gpsimd)
                emit_dma(1, 0, nc.sync)
                ohs0 = emit_oh(0)
                emit_batch(0, ohs0, SQ_ENG_B0)
                # b1 reuses b0's buffers as the wq3 matmuls retire them
                # fc-major: fc3 (Pool, slowest chain) gets the first-freed
                xts[(1, 3)] = xp.tile([H, FC, W], F32R, name="xt")
                xts[(1, 2)] = xp.tile([H, FC, W], F32R, name="xt")
                xts[(1, 1)] = xp.tile([H, FC, W], F32R, name="xt")
                emit_dma(1, 3, nc.gpsimd)
                emit_dma(1, 2, nc.scalar)
                emit_dma(1, 1, nc.sync)
                ohs1 = emit_oh(1)
                emit_batch(1, ohs1, SQ_ENG_B1)

            # combine diagonal blocks:
            #   protos[c, fc*FC + f] = sum_wl psq[fc][wl*CP+c, f*QW+wl]
            protos_sb = finp.tile([C, F], F32)
            for fc in range(NFC):
                pv = psq[fc][:].rearrange("m (f w) -> m f w", w=QW)
                dst = protos_sb[:, fc * FC : (fc + 1) * FC]
                nc.scalar.copy(out=dst, in_=pv[0:C, :, 0])
                for wl in range(1, QW):
                    nc.vector.tensor_add(dst, dst, pv[wl * CP : wl * CP + C, :, wl])
            if not _NO_CC:
                nc.sync.dma_start(out=cc_in.ap(), in_=protos_sb)
                nc.gpsimd.collective_compute(
                    "AllReduce",
                    OP.add,
                    ins=[cc_in.ap().opt()],
                    outs=[cc_out.ap().opt()],
                    replica_groups=[list(range(NCORES))],
                )
                red_sb = finp.tile([C, F], F32)
                nc.sync.dma_start(out=red_sb, in_=cc_out.ap())
            else:
                red_sb = protos_sb

            # column norms (per class over F): pn2[c] = sum_f red[c,f]^2
            scr = finp.tile([C, F], F32)
            pn = finp.tile([C, 1], F32)
            nc.vector.tensor_mul(scr, red_sb, red_sb)
            nc.vector.tensor_reduce(out=pn, in_=scr, axis=AX.X, op=OP.add)
            nc.scalar.activation(out=pn, in_=pn, func=ACT_FN.Sqrt, bias=eps_sb[:C])
            pninv = finp.tile([C, 1], F32)
            nc.vector.reciprocal(out=pninv, in_=pn)
            nc.vector.tensor_scalar_mul(out=red_sb, in0=red_sb, scalar1=pninv)

            # store [C-1, F]; host transposes to [F, C-1]
            nc.sync.dma_start(out=out_d.ap(), in_=red_sb[0 : C - 1, :])

    nc.compile()
    return nc


_NC_CACHE = None


def _get_nc():
    global _NC_CACHE
    if _NC_CACHE is None:
        _NC_CACHE = build_nc()
    return _NC_CACHE


def kernel(features: np.ndarray, labels: np.ndarray) -> np.ndarray:
    features = np.ascontiguousarray(np.asarray(features, dtype=np.float32))
    # label values 0..19 are exact in fp16
    labs_f16 = np.asarray(labels, dtype=np.float16)

    nc = _get_nc()
    in_maps = []
    for core in range(NCORES):
        in_maps.append(
            {
                "feats": features[core * BPC : (core + 1) * BPC],
                "labs": np.ascontiguousarray(labs_f16[core * BPC : (core + 1) * BPC]),
            }
        )
    res = run_bass_kernel_spmd(nc, in_maps, core_ids=list(range(NCORES)))
    out = np.asarray(res.results[0]["out"], dtype=np.float32)  # [C-1, F]
    return np.ascontiguousarray(out.T)  # [F, C-1]
